# revision 11
# baseline (speedup 1.0000x reference)
"""Trainium2 Bass kernel for ConstructAdjMatrixWithHomogeneous — single launch.

out = I + D^-1/2 @ adj @ D^-1/2,  adj = [[C, A], [A^T, Dd]],
C = filtered_cell_kernel [4000,4000], Dd = filtered_drug_sim [4000,4000],
A = original_cell_drug_adj [4000,4000]; deg = rowsum(adj)+eps, d = deg**-0.5.

Sharding (8 cores): 128-aligned row bands. Core c owns rows
[512c, 512c+512) of each matrix; core 7 owns rows 3584:4000 plus 96
zero-padded junk rows so every DMA tile is a full [128,4000]
(partial-partition DMAs measured 3x slower per byte on this part).

Single launch per core, load order A -> D -> C so the drug-side
dependency chain resolves while C is still loading:
  - A bands: rowsums (DVE) + column-sum partials via bf16 ones-matmul
    (bf16 convert per 512-chunk; 4x faster PE than fp32, error ~1e-4
    on a 4000-term sum). Partials ReduceScatter(add) so each core gets
    its own 512-row drug-degree chunk back (~t=30us).
  - D bands: rowsums; drug deg = rs_d + RS chunk (PE-transposed to
    [128,4]); d = sqrt(reciprocal(deg+eps)) (ACT Rsqrt is banned);
    AllGather(drug d) fires ~t=50us.
  - C bands: rowsums; cell deg = rs_c + rs_a local; AllGather(cell d)
    fires ~t=72us right after the last load.
  Phase B (per 512-col chunk: PE ones-row matmul broadcasts the
  gathered d line into PSUM, then ONE fused in-place DVE op
  scalar_tensor_tensor(x, x, d_row, bc) = (x*d_row)*d_col):
  BR first (only needs drug d, ~t=60), then TR, then ats = PE
  transposes of the scaled A tiles, TL last (cell d arrives ~t=82).

Ring discipline (measured): one HWDGE ring sustains ~317 GB/s, two
directions on separate rings ~420 GB/s aggregate, mixed directions on
a ring much worse. Sync ring: all input loads, then BR + ats stores.
Scalar ring: collective bounce + d-chunk traffic, then TR + TL stores.

The +I is added on the host during assembly (O(N) work).
"""
import sys

sys.path.insert(0, "/opt/trn_rl_repo")

import json
import numpy as np

import concourse.bass as bass
import concourse.mybir as mybir
import concourse.tile as tile
import concourse.bass2jax as bass2jax
from concourse.alu_op_type import AluOpType
from concourse.bass_utils import run_bass_kernel_spmd, compile_bir_kernel

F32 = mybir.dt.float32
BF16 = mybir.dt.bfloat16
NCORES = 8
PB = 128               # partition band size
NBAND = 4
CR = PB * NBAND        # 512 rows of each matrix per core
NMAT = 4000
N = 8000
EPS = np.float32(1e-9)
NCH = 8                # 512-wide column chunks; last is 416
CHW = [512] * 7 + [416]
NFULL = NMAT // PB     # 31 full transpose chunks
TAIL = NMAT - NFULL * PB  # 32

# ---------------------------------------------------------------------------
# Walrus workaround: this toolchain only supports ONE sync-wait condition per
# instruction ("Too many sync wait commands" in CoreV3GenImpl otherwise).
# Split any instruction carrying >1 waits into preceding NoOps, 1 wait each.
# ---------------------------------------------------------------------------
_MAXW = 1


def _split_waits_bytes(bir_bytes):
    bir = json.loads(bir_bytes)
    n_new = 0
    for fn in bir["functions"]:
        for blk in fn["blocks"]:
            insts = blk.get("instructions", [])
            out = []
            for ins in insts:
                si = ins.get("sync_info") or {}
                waits = si.get("on_wait") or []
                while len(waits) > _MAXW:
                    chunk, waits = waits[:_MAXW], waits[_MAXW:]
                    n_new += 1
                    out.append({
                        "name": ins["name"] + f"_ws{n_new}",
                        "opcode": "NoOp",
                        "engine": ins["engine"],
                        "ins": [], "outs": [],
                        "sync_info": {"on_update": [], "on_wait": chunk},
                    })
                si["on_wait"] = waits
                ins["sync_info"] = si
                out.append(ins)
            blk["instructions"] = out
    return json.dumps(bir).encode()


def _patched_compile_bir_kernel(bir_json, tmpdir, neff_name="file.neff"):
    return compile_bir_kernel(_split_waits_bytes(bir_json), tmpdir,
                              neff_name=neff_name)


bass2jax.compile_bir_kernel = _patched_compile_bir_kernel


def _build(reps=1, no_coll=False, stage="full", timing_mode=False):
    nc = bass.Bass(num_devices=NCORES)
    cb = nc.declare_dram_parameter("cb", [CR, NMAT], F32, isOutput=False)
    ab = nc.declare_dram_parameter("ab", [CR, NMAT], F32, isOutput=False)
    db = nc.declare_dram_parameter("db", [CR, NMAT], F32, isOutput=False)
    if no_coll:
        rs_out_h = nc.declare_dram_parameter("rs_out_h", [1, 512], F32,
                                             isOutput=False)
        agc_out_h = nc.declare_dram_parameter("agc_out_h", [1, 4096], F32,
                                              isOutput=False)
        agd_out_h = nc.declare_dram_parameter("agd_out_h", [1, 4096], F32,
                                              isOutput=False)
    if timing_mode:
        # Identical device work, but big results land in Internal DRAM
        # scratch so the PJRT output plumbing (which costs ~0.5ms/MB per
        # call and jitters) stays tiny. One real [1,1] output remains.
        ok = nc.declare_dram_parameter("ok", [1, 1], F32, isOutput=True)
    else:
        top = nc.declare_dram_parameter("top", [CR, N], F32, isOutput=True)
        br = nc.declare_dram_parameter("br", [CR, NMAT], F32, isOutput=True)
        ats = nc.declare_dram_parameter("ats", [NMAT, CR], F32, isOutput=True)

    ident = nc.inline_tensor(np.eye(PB, dtype=np.float32), name="ident128")
    Sqrt = mybir.ActivationFunctionType.Sqrt
    Copy = mybir.ActivationFunctionType.Copy
    RG = [list(range(NCORES))]

    with tile.TileContext(nc) as tc:
        with (
            tc.tile_pool(name="const", bufs=1) as const,
            tc.tile_pool(name="data", bufs=1) as data,
            tc.tile_pool(name="small", bufs=1) as small,
            tc.tile_pool(name="stg", bufs=2) as stg,
            tc.tile_pool(name="dstg", bufs=2) as dstg,
            tc.tile_pool(name="att", bufs=2) as att,
            tc.tile_pool(name="bfp", bufs=2) as bfp,
            tc.tile_pool(name="ps", bufs=1, space="PSUM") as ps,
            tc.tile_pool(name="dram", bufs=1, space="DRAM") as dram,
        ):
            idt = const.tile([PB, PB], F32, name="idt")
            nc.sync.dma_start(idt[:], ident[:])
            ones_bf = const.tile([PB, 1], BF16, name="ones_bf")
            nc.gpsimd.memset(ones_bf[:], 1.0)
            ones_row = const.tile([1, PB], F32, name="ones_row")
            nc.gpsimd.memset(ones_row[:], 1.0)
            ones1 = const.tile([1, 1], F32, name="ones1")
            nc.gpsimd.memset(ones1[:], 1.0)
            if timing_mode:
                top = dram.tile([CR, N], F32, tag="top_s", name="top_s")
                br = dram.tile([CR, NMAT], F32, tag="br_s", name="br_s")
                ats = dram.tile([NMAT, CR], F32, tag="ats_s", name="ats_s")

            for _ in range(reps):
                rs_in = dram.tile([1, 4096], F32, tag="rs_in", name="rs_in")
                rs_out = dram.tile([1, 512], F32, tag="rs_out", name="rs_out")
                agc_in = dram.tile([1, 512], F32, tag="agc_in", name="agc_in")
                agc_out = dram.tile([1, 4096], F32, tag="agc_out", name="agc_out")
                agd_in = dram.tile([1, 512], F32, tag="agd_in", name="agd_in")
                agd_out = dram.tile([1, 4096], F32, tag="agd_out", name="agd_out")

                rs_a = small.tile([PB, NBAND], F32, tag="rs_a", name="rs_a")
                rs_c = small.tile([PB, NBAND], F32, tag="rs_c", name="rs_c")
                rs_d = small.tile([PB, NBAND], F32, tag="rs_d", name="rs_d")
                deg_c = small.tile([PB, NBAND], F32, tag="deg_c", name="deg_c")
                deg_d = small.tile([PB, NBAND], F32, tag="deg_d", name="deg_d")
                drow_c = small.tile([PB, NBAND], F32, tag="drow_c", name="drow_c")
                drow_d = small.tile([PB, NBAND], F32, tag="drow_d", name="drow_d")

                # ---- A bands: rowsums + bf16 colsum partials ----
                cs_t = [ps.tile([1, 1024], F32, tag=f"q{i}", name=f"cst{i}")
                        for i in range(4)]
                cs_ps = [cs_t[j // 2][0:1, (j % 2) * 512:(j % 2) * 512 + 512]
                         for j in range(NCH)]
                ta = []
                for b in range(NBAND):
                    t = data.tile([PB, NMAT], F32, tag=f"a{b}", name="ta")
                    nc.sync.dma_start(t[:], ab[b * PB:(b + 1) * PB, :])
                    nc.scalar.activation(t[:], t[:], Copy,
                                         accum_out=rs_a[:, b:b + 1])
                    for j in range(NCH):
                        w = CHW[j]
                        xbf = bfp.tile([PB, 512], BF16, tag="bf", name="xbf")
                        nc.gpsimd.tensor_copy(xbf[:, :w],
                                              t[:, 512 * j:512 * j + w])
                        nc.tensor.matmul(cs_ps[j][:, :w], ones_bf[:],
                                         xbf[:, :w],
                                         start=(b == 0), stop=(b == NBAND - 1))
                    ta.append(t)

                for j in range(NCH):
                    w = CHW[j]
                    cst = stg.tile([1, 512], F32, tag="cs_stg", name="cst")
                    nc.scalar.copy(cst[:, :w], cs_ps[j][:, :w])
                    nc.scalar.dma_start(rs_in[0:1, 512 * j:512 * j + w],
                                        cst[:, :w])
                if no_coll:
                    rs_out = rs_out_h
                else:
                    nc.gpsimd.collective_compute(
                        "ReduceScatter", AluOpType.add, replica_groups=RG,
                        ins=[rs_in.opt()], outs=[rs_out.opt()])

                # ---- D bands: rowsums -> drug d -> AllGather(drug) ----
                td = []
                for b in range(NBAND):
                    t = data.tile([PB, NMAT], F32, tag=f"d{b}", name="td")
                    nc.sync.dma_start(t[:], db[b * PB:(b + 1) * PB, :])
                    nc.scalar.activation(t[:], t[:], Copy,
                                         accum_out=rs_d[:, b:b + 1])
                    td.append(t)
                rso = stg.tile([1, 512], F32, tag="rso", bufs=1, name="rso")
                nc.scalar.dma_start(rso[:], rs_out[:])
                ptcs = ps.tile([PB, NBAND], F32, tag="q0", name="ptcs")
                for b in range(NBAND):
                    nc.tensor.transpose(ptcs[:, b:b + 1],
                                        rso[0:1, PB * b:PB * (b + 1)],
                                        ones1[:])
                nc.vector.tensor_add(deg_d[:], rs_d[:], ptcs[:])
                nc.vector.tensor_scalar_add(deg_d[:], deg_d[:], float(EPS))
                nc.vector.reciprocal(drow_d[:], deg_d[:])
                nc.scalar.activation(drow_d[:], drow_d[:], Sqrt)
                ptd = ps.tile([1, 512], F32, tag="q1", name="ptd")
                for b in range(NBAND):
                    nc.tensor.transpose(ptd[0:1, PB * b:PB * (b + 1)],
                                        drow_d[:, b:b + 1], idt[:])
                agds = stg.tile([1, 512], F32, tag="ag_stg", bufs=1, name="agds")
                nc.scalar.copy(agds[:], ptd[:])
                nc.scalar.dma_start(agd_in[:], agds[:])
                if no_coll:
                    agd_out = agd_out_h
                else:
                    nc.gpsimd.collective_compute(
                        "AllGather", AluOpType.bypass, replica_groups=RG,
                        ins=[agd_in.opt()], outs=[agd_out.opt()])

                # ---- C bands: rowsums -> cell d -> AllGather(cell) ----
                tcl = []
                for b in range(NBAND):
                    t = data.tile([PB, NMAT], F32, tag=f"c{b}", name="tcl")
                    nc.sync.dma_start(t[:], cb[b * PB:(b + 1) * PB, :])
                    nc.scalar.activation(t[:], t[:], Copy,
                                         accum_out=rs_c[:, b:b + 1])
                    tcl.append(t)
                nc.vector.tensor_add(deg_c[:], rs_c[:], rs_a[:])
                nc.vector.tensor_scalar_add(deg_c[:], deg_c[:], float(EPS))
                nc.vector.reciprocal(drow_c[:], deg_c[:])
                nc.scalar.activation(drow_c[:], drow_c[:], Sqrt)
                ptc = ps.tile([1, 512], F32, tag="q2", name="ptc")
                for b in range(NBAND):
                    nc.tensor.transpose(ptc[0:1, PB * b:PB * (b + 1)],
                                        drow_c[:, b:b + 1], idt[:])
                agcs = stg.tile([1, 512], F32, tag="ag_stg", bufs=1, name="agcs")
                nc.scalar.copy(agcs[:], ptc[:])
                nc.scalar.dma_start(agc_in[:], agcs[:])
                if no_coll:
                    agc_out = agc_out_h
                else:
                    nc.gpsimd.collective_compute(
                        "AllGather", AluOpType.bypass, replica_groups=RG,
                        ins=[agc_in.opt()], outs=[agc_out.opt()])

                if stage == "phaseA":
                    continue

                # ---- phase B. BR first (drug d only, earliest ready) ----
                def scale_block(tiles, drow, line, tags):
                    for k in range(4):
                        w2 = 1024 if k < 3 else 928
                        sl = slice(1024 * k, 1024 * k + w2)
                        bc = ps.tile([PB, 1024], F32, tag=tags[k % 2],
                                     name="bc")
                        for h in range(2):
                            j = 2 * k + h
                            w = CHW[j]
                            dch = dstg.tile([1, 512], F32, tag="dch",
                                            name="dch")
                            nc.scalar.dma_start(dch[:, :w],
                                                line[0:1,
                                                     512 * j:512 * j + w])
                            nc.tensor.matmul(bc[:, 512 * h:512 * h + w],
                                             ones_row[:], dch[0:1, :w],
                                             start=True, stop=True)
                        for b in range(NBAND):
                            nc.vector.scalar_tensor_tensor(
                                tiles[b][:, sl], tiles[b][:, sl],
                                drow[:, b:b + 1], bc[:, :w2],
                                AluOpType.mult, AluOpType.mult)

                scale_block(td, drow_d, agd_out, ("q0", "q1"))
                for b in range(NBAND):
                    nc.scalar.dma_start(br[b * PB:(b + 1) * PB, :], td[b][:])

                # ---- TR = dcell * A * ddrug ----
                scale_block(ta, drow_c, agd_out, ("q2", "q3"))
                for b in range(NBAND):
                    nc.scalar.dma_start(top[b * PB:(b + 1) * PB, NMAT:],
                                        ta[b][:])

                # ---- TL = dcell * C * dcell ----
                scale_block(tcl, drow_c, agc_out, ("q0", "q1"))

                if stage == "scale":
                    for b in range(NBAND):
                        nc.scalar.dma_start(top[b * PB:(b + 1) * PB, 0:NMAT],
                                            tcl[b][:])
                    continue

                # ---- ats = (scaled A)^T; TL stores last on scalar ----
                for c in range(NFULL + 1):
                    cw = PB if c < NFULL else TAIL
                    pt = ps.tile([PB, 512], F32,
                                 tag=("q2", "q3")[c % 2], name="pt")
                    for b in range(NBAND):
                        nc.tensor.transpose(
                            pt[:cw, b * PB:(b + 1) * PB],
                            ta[b][:, c * PB:c * PB + cw], idt[:])
                    at_sb = att.tile([PB, 512], F32, tag="att", name="at_sb")
                    nc.scalar.copy(at_sb[:cw, :], pt[:cw, :])
                    nc.sync.dma_start(ats[c * PB:c * PB + cw, :],
                                      at_sb[:cw, :])
                for b in range(NBAND):
                    nc.scalar.dma_start(top[b * PB:(b + 1) * PB, 0:NMAT],
                                        tcl[b][:])
                if timing_mode:
                    nc.scalar.dma_start(ok[:], drow_c[0:1, 0:1])
    return nc


_programs_cache = {}


def _program():
    if "l" not in _programs_cache:
        _programs_cache["l"] = _build()
    return _programs_cache["l"]


def _make_in_maps(C, A, D):
    in_maps = []
    for c in range(NCORES):
        s = 512 * c
        e = min(s + CR, NMAT)
        if e - s == CR:
            in_maps.append({"cb": C[s:e], "ab": A[s:e], "db": D[s:e]})
        else:
            m = {}
            for name, M in (("cb", C), ("ab", A), ("db", D)):
                t = np.zeros((CR, NMAT), dtype=np.float32)
                t[:e - s] = M[s:e]
                m[name] = t
            in_maps.append(m)
    return in_maps


def kernel(filtered_cell_kernel, filtered_drug_sim, original_cell_drug_adj,
           enable_homogeneous_graph):
    C = np.ascontiguousarray(np.asarray(filtered_cell_kernel, dtype=np.float32))
    D = np.ascontiguousarray(np.asarray(filtered_drug_sim, dtype=np.float32))
    A = np.ascontiguousarray(np.asarray(original_cell_drug_adj, dtype=np.float32))
    enable = int(np.asarray(enable_homogeneous_graph))
    if not enable:
        C = np.zeros_like(C)
        D = np.zeros_like(D)

    r = run_bass_kernel_spmd(_program(), _make_in_maps(C, A, D),
                             core_ids=list(range(NCORES))).results

    out = np.empty((N, N), dtype=np.float32)
    for c in range(NCORES):
        s = 512 * c
        e = min(s + CR, NMAT)
        n = e - s
        out[s:e, :] = r[c]["top"][:n]
        out[NMAT + s:NMAT + e, NMAT:] = r[c]["br"][:n]
        out[NMAT:, s:e] = r[c]["ats"][:, :n]
    idx = np.arange(N)
    out[idx, idx] += np.float32(1.0)
    return out


# revision 13
# speedup vs baseline: 1.1460x; 1.1460x over previous
"""Trainium2 Bass kernel for ConstructAdjMatrixWithHomogeneous — single launch.

out = I + D^-1/2 @ adj @ D^-1/2,  adj = [[C, A], [A^T, Dd]],
C = filtered_cell_kernel [4000,4000], Dd = filtered_drug_sim [4000,4000],
A = original_cell_drug_adj [4000,4000]; deg = rowsum(adj)+eps, d = deg**-0.5.

Sharding (8 cores): 128-aligned row bands. Core c owns rows
[512c, 512c+512) of each matrix; core 7 owns rows 3584:4000 plus 96
zero-padded junk rows so every DMA tile is a full [128,4000]
(partial-partition DMAs measured 3x slower per byte on this part).

Single launch per core, load order A -> D -> C so the drug-side
dependency chain resolves while C is still loading:
  - A bands: rowsums (DVE) + column-sum partials via bf16 ones-matmul
    (bf16 convert per 512-chunk; 4x faster PE than fp32, error ~1e-4
    on a 4000-term sum). Partials ReduceScatter(add) so each core gets
    its own 512-row drug-degree chunk back (~t=30us).
  - D bands: rowsums; drug deg = rs_d + RS chunk (PE-transposed to
    [128,4]); d = sqrt(reciprocal(deg+eps)) (ACT Rsqrt is banned);
    AllGather(drug d) fires ~t=50us.
  - C bands: rowsums; cell deg = rs_c + rs_a local; AllGather(cell d)
    fires ~t=72us right after the last load.
  Phase B (per 512-col chunk: PE ones-row matmul broadcasts the
  gathered d line into PSUM, then ONE fused in-place DVE op
  scalar_tensor_tensor(x, x, d_row, bc) = (x*d_row)*d_col):
  BR first (only needs drug d, ~t=60), then TR, then ats = PE
  transposes of the scaled A tiles, TL last (cell d arrives ~t=82).

Ring discipline (measured): one HWDGE ring sustains ~317 GB/s, two
directions on separate rings ~420 GB/s aggregate, mixed directions on
a ring much worse. Sync ring: all input loads, then BR + ats stores.
Scalar ring: collective bounce + d-chunk traffic, then TR + TL stores.

The +I is added on the host during assembly (O(N) work).
"""
import sys

sys.path.insert(0, "/opt/trn_rl_repo")

import json
import numpy as np

import concourse.bass as bass
import concourse.mybir as mybir
import concourse.tile as tile
import concourse.bass2jax as bass2jax
from concourse.alu_op_type import AluOpType
from concourse.bass_utils import run_bass_kernel_spmd, compile_bir_kernel

F32 = mybir.dt.float32
BF16 = mybir.dt.bfloat16
NCORES = 8
PB = 128               # partition band size
NBAND = 4
CR = PB * NBAND        # 512 rows of each matrix per core
NMAT = 4000
N = 8000
EPS = np.float32(1e-9)
NCH = 8                # 512-wide column chunks; last is 416
CHW = [512] * 7 + [416]
NFULL = NMAT // PB     # 31 full transpose chunks
TAIL = NMAT - NFULL * PB  # 32

# ---------------------------------------------------------------------------
# Walrus workaround: this toolchain only supports ONE sync-wait condition per
# instruction ("Too many sync wait commands" in CoreV3GenImpl otherwise).
# Split any instruction carrying >1 waits into preceding NoOps, 1 wait each.
# ---------------------------------------------------------------------------
_MAXW = 1


def _split_waits_bytes(bir_bytes):
    bir = json.loads(bir_bytes)
    n_new = 0
    for fn in bir["functions"]:
        for blk in fn["blocks"]:
            insts = blk.get("instructions", [])
            out = []
            for ins in insts:
                si = ins.get("sync_info") or {}
                waits = si.get("on_wait") or []
                while len(waits) > _MAXW:
                    chunk, waits = waits[:_MAXW], waits[_MAXW:]
                    n_new += 1
                    out.append({
                        "name": ins["name"] + f"_ws{n_new}",
                        "opcode": "NoOp",
                        "engine": ins["engine"],
                        "ins": [], "outs": [],
                        "sync_info": {"on_update": [], "on_wait": chunk},
                    })
                si["on_wait"] = waits
                ins["sync_info"] = si
                out.append(ins)
            blk["instructions"] = out
    return json.dumps(bir).encode()


def _patched_compile_bir_kernel(bir_json, tmpdir, neff_name="file.neff"):
    return compile_bir_kernel(_split_waits_bytes(bir_json), tmpdir,
                              neff_name=neff_name)


bass2jax.compile_bir_kernel = _patched_compile_bir_kernel


def _build(reps=1, no_coll=False, stage="full", timing_mode=False):
    nc = bass.Bass(num_devices=NCORES)
    cb = nc.declare_dram_parameter("cb", [CR, NMAT], F32, isOutput=False)
    ab = nc.declare_dram_parameter("ab", [CR, NMAT], F32, isOutput=False)
    db = nc.declare_dram_parameter("db", [CR, NMAT], F32, isOutput=False)
    if no_coll:
        rs_out_h = nc.declare_dram_parameter("rs_out_h", [1, 512], F32,
                                             isOutput=False)
        agc_out_h = nc.declare_dram_parameter("agc_out_h", [1, 4096], F32,
                                              isOutput=False)
        agd_out_h = nc.declare_dram_parameter("agd_out_h", [1, 4096], F32,
                                              isOutput=False)
    if timing_mode:
        # Identical device work, but big results land in Internal DRAM
        # scratch so the PJRT output plumbing (which costs ~0.5ms/MB per
        # call and jitters) stays tiny. One real [1,1] output remains.
        ok = nc.declare_dram_parameter("ok", [1, 1], F32, isOutput=True)
    else:
        top = nc.declare_dram_parameter("top", [CR, N], F32, isOutput=True)
        br = nc.declare_dram_parameter("br", [CR, NMAT], F32, isOutput=True)
        ats = nc.declare_dram_parameter("ats", [NMAT, CR], F32, isOutput=True)

    ident = nc.inline_tensor(np.eye(PB, dtype=np.float32), name="ident128")
    Sqrt = mybir.ActivationFunctionType.Sqrt
    Copy = mybir.ActivationFunctionType.Copy
    RG = [list(range(NCORES))]

    with tile.TileContext(nc) as tc:
        with (
            tc.tile_pool(name="const", bufs=1) as const,
            tc.tile_pool(name="data", bufs=1) as data,
            tc.tile_pool(name="small", bufs=1) as small,
            tc.tile_pool(name="stg", bufs=2) as stg,
            tc.tile_pool(name="dstg", bufs=2) as dstg,
            tc.tile_pool(name="att", bufs=2) as att,
            tc.tile_pool(name="bfp", bufs=2) as bfp,
            tc.tile_pool(name="ps", bufs=1, space="PSUM") as ps,
            tc.tile_pool(name="dram", bufs=1, space="DRAM") as dram,
        ):
            idt = const.tile([PB, PB], F32, name="idt")
            nc.sync.dma_start(idt[:], ident[:])
            ones_bf = const.tile([PB, 1], BF16, name="ones_bf")
            nc.gpsimd.memset(ones_bf[:], 1.0)
            ones_row = const.tile([1, PB], F32, name="ones_row")
            nc.gpsimd.memset(ones_row[:], 1.0)
            ones1 = const.tile([1, 1], F32, name="ones1")
            nc.gpsimd.memset(ones1[:], 1.0)
            if timing_mode:
                top = dram.tile([CR, N], F32, tag="top_s", name="top_s")
                br = dram.tile([CR, NMAT], F32, tag="br_s", name="br_s")
                ats = dram.tile([NMAT, CR], F32, tag="ats_s", name="ats_s")

            for _ in range(reps):
                rs_in = dram.tile([1, 4096], F32, tag="rs_in", name="rs_in")
                rs_out = dram.tile([1, 512], F32, tag="rs_out", name="rs_out")
                agc_in = dram.tile([1, 512], F32, tag="agc_in", name="agc_in")
                agc_out = dram.tile([1, 4096], F32, tag="agc_out", name="agc_out")
                agd_in = dram.tile([1, 512], F32, tag="agd_in", name="agd_in")
                agd_out = dram.tile([1, 4096], F32, tag="agd_out", name="agd_out")

                rs_a = small.tile([PB, NBAND], F32, tag="rs_a", name="rs_a")
                rs_c = small.tile([PB, NBAND], F32, tag="rs_c", name="rs_c")
                rs_d = small.tile([PB, NBAND], F32, tag="rs_d", name="rs_d")
                deg_c = small.tile([PB, NBAND], F32, tag="deg_c", name="deg_c")
                deg_d = small.tile([PB, NBAND], F32, tag="deg_d", name="deg_d")
                drow_c = small.tile([PB, NBAND], F32, tag="drow_c", name="drow_c")
                drow_d = small.tile([PB, NBAND], F32, tag="drow_d", name="drow_d")

                # ---- A bands: rowsums + bf16 colsum partials ----
                cs_t = [ps.tile([1, 1024], F32, tag=f"q{i}", name=f"cst{i}")
                        for i in range(4)]
                cs_ps = [cs_t[j // 2][0:1, (j % 2) * 512:(j % 2) * 512 + 512]
                         for j in range(NCH)]
                ta = []
                for b in range(NBAND):
                    t = data.tile([PB, NMAT], F32, tag=f"a{b}", name="ta")
                    nc.sync.dma_start(t[:], ab[b * PB:(b + 1) * PB, :])
                    nc.vector.reduce_sum(rs_a[:, b:b + 1], t[:],
                                         axis=mybir.AxisListType.X)
                    for j in range(NCH):
                        w = CHW[j]
                        xbf = bfp.tile([PB, 512], BF16, tag="bf", name="xbf")
                        if j < 5:
                            nc.gpsimd.tensor_copy(xbf[:, :w],
                                                  t[:, 512 * j:512 * j + w])
                        else:
                            nc.scalar.activation(xbf[:, :w],
                                                 t[:, 512 * j:512 * j + w],
                                                 Copy)
                        nc.tensor.matmul(cs_ps[j][:, :w], ones_bf[:],
                                         xbf[:, :w],
                                         start=(b == 0), stop=(b == NBAND - 1))
                    ta.append(t)

                for j in range(NCH):
                    w = CHW[j]
                    cst = stg.tile([1, 512], F32, tag="cs_stg", name="cst")
                    nc.scalar.copy(cst[:, :w], cs_ps[j][:, :w])
                    nc.scalar.dma_start(rs_in[0:1, 512 * j:512 * j + w],
                                        cst[:, :w])
                if no_coll:
                    rs_out = rs_out_h
                else:
                    nc.gpsimd.collective_compute(
                        "ReduceScatter", AluOpType.add, replica_groups=RG,
                        ins=[rs_in.opt()], outs=[rs_out.opt()])

                # ---- D bands: rowsums -> drug d -> AllGather(drug) ----
                td = []
                for b in range(NBAND):
                    t = data.tile([PB, NMAT], F32, tag=f"d{b}", name="td")
                    nc.sync.dma_start(t[:], db[b * PB:(b + 1) * PB, :])
                    nc.vector.reduce_sum(rs_d[:, b:b + 1], t[:],
                                         axis=mybir.AxisListType.X)
                    td.append(t)
                rso = stg.tile([1, 512], F32, tag="rso", bufs=1, name="rso")
                nc.scalar.dma_start(rso[:], rs_out[:])
                ptcs = ps.tile([PB, NBAND], F32, tag="q0", name="ptcs")
                for b in range(NBAND):
                    nc.tensor.transpose(ptcs[:, b:b + 1],
                                        rso[0:1, PB * b:PB * (b + 1)],
                                        ones1[:])
                nc.vector.tensor_add(deg_d[:], rs_d[:], ptcs[:])
                nc.vector.tensor_scalar_add(deg_d[:], deg_d[:], float(EPS))
                nc.vector.reciprocal(drow_d[:], deg_d[:])
                nc.scalar.activation(drow_d[:], drow_d[:], Sqrt)
                ptd = ps.tile([1, 512], F32, tag="q1", name="ptd")
                for b in range(NBAND):
                    nc.tensor.transpose(ptd[0:1, PB * b:PB * (b + 1)],
                                        drow_d[:, b:b + 1], idt[:])
                agds = stg.tile([1, 512], F32, tag="ag_stg", bufs=1, name="agds")
                nc.scalar.copy(agds[:], ptd[:])
                nc.scalar.dma_start(agd_in[:], agds[:])
                if no_coll:
                    agd_out = agd_out_h
                else:
                    nc.gpsimd.collective_compute(
                        "AllGather", AluOpType.bypass, replica_groups=RG,
                        ins=[agd_in.opt()], outs=[agd_out.opt()])

                # ---- C bands: rowsums -> cell d -> AllGather(cell) ----
                tcl = []
                for b in range(NBAND):
                    t = data.tile([PB, NMAT], F32, tag=f"c{b}", name="tcl")
                    nc.sync.dma_start(t[:], cb[b * PB:(b + 1) * PB, :])
                    nc.vector.reduce_sum(rs_c[:, b:b + 1], t[:],
                                         axis=mybir.AxisListType.X)
                    tcl.append(t)
                nc.vector.tensor_add(deg_c[:], rs_c[:], rs_a[:])
                nc.vector.tensor_scalar_add(deg_c[:], deg_c[:], float(EPS))
                nc.vector.reciprocal(drow_c[:], deg_c[:])
                nc.scalar.activation(drow_c[:], drow_c[:], Sqrt)
                ptc = ps.tile([1, 512], F32, tag="q2", name="ptc")
                for b in range(NBAND):
                    nc.tensor.transpose(ptc[0:1, PB * b:PB * (b + 1)],
                                        drow_c[:, b:b + 1], idt[:])
                agcs = stg.tile([1, 512], F32, tag="ag_stg", bufs=1, name="agcs")
                nc.scalar.copy(agcs[:], ptc[:])
                nc.scalar.dma_start(agc_in[:], agcs[:])
                if no_coll:
                    agc_out = agc_out_h
                else:
                    nc.gpsimd.collective_compute(
                        "AllGather", AluOpType.bypass, replica_groups=RG,
                        ins=[agc_in.opt()], outs=[agc_out.opt()])

                if stage == "phaseA":
                    continue

                # ---- phase B. BR first (drug d only, earliest ready) ----
                def scale_block(tiles, drow, line, tags, chunk_done=None):
                    for k in range(4):
                        w2 = 1024 if k < 3 else 928
                        sl = slice(1024 * k, 1024 * k + w2)
                        bc = ps.tile([PB, 1024], F32, tag=tags[k % 2],
                                     name="bc")
                        for h in range(2):
                            j = 2 * k + h
                            w = CHW[j]
                            dch = dstg.tile([1, 512], F32, tag="dch",
                                            name="dch")
                            nc.scalar.dma_start(dch[:, :w],
                                                line[0:1,
                                                     512 * j:512 * j + w])
                            nc.tensor.matmul(bc[:, 512 * h:512 * h + w],
                                             ones_row[:], dch[0:1, :w],
                                             start=True, stop=True)
                        for b in range(NBAND):
                            nc.vector.scalar_tensor_tensor(
                                tiles[b][:, sl], tiles[b][:, sl],
                                drow[:, b:b + 1], bc[:, :w2],
                                AluOpType.mult, AluOpType.mult)
                        if chunk_done is not None:
                            chunk_done(k)

                scale_block(td, drow_d, agd_out, ("q0", "q1"))
                for b in range(NBAND):
                    nc.scalar.dma_start(br[b * PB:(b + 1) * PB, :], td[b][:])

                # ---- TR = dcell * A * ddrug; transpose each chunk as it
                # is scaled so the PE starts the ats slab early ----
                def tr_chunk_done(k):
                    c0 = 8 * k
                    c1 = min(8 * k + 8, NFULL + 1)
                    for c in range(c0, c1):
                        cw = PB if c < NFULL else TAIL
                        pt = ps.tile([PB, 512], F32,
                                     tag=("q0", "q1")[c % 2], name="pt")
                        for b in range(NBAND):
                            nc.tensor.transpose(
                                pt[:cw, b * PB:(b + 1) * PB],
                                ta[b][:, c * PB:c * PB + cw], idt[:])
                        at_sb = att.tile([PB, 512], F32, tag="att",
                                         name="at_sb")
                        nc.scalar.copy(at_sb[:cw, :], pt[:cw, :])
                        nc.sync.dma_start(ats[c * PB:c * PB + cw, :],
                                          at_sb[:cw, :])

                scale_block(ta, drow_c, agd_out, ("q2", "q3"),
                            chunk_done=tr_chunk_done)
                for b in range(NBAND):
                    nc.scalar.dma_start(top[b * PB:(b + 1) * PB, NMAT:],
                                        ta[b][:])

                # ---- TL = dcell * C * dcell ----
                scale_block(tcl, drow_c, agc_out, ("q2", "q3"))

                if stage == "scale":
                    for b in range(NBAND):
                        nc.scalar.dma_start(top[b * PB:(b + 1) * PB, 0:NMAT],
                                            tcl[b][:])
                    continue

                for b in range(NBAND):
                    nc.scalar.dma_start(top[b * PB:(b + 1) * PB, 0:NMAT],
                                        tcl[b][:])
                if timing_mode:
                    nc.scalar.dma_start(ok[:], drow_c[0:1, 0:1])
    return nc


_programs_cache = {}


def _program():
    if "l" not in _programs_cache:
        _programs_cache["l"] = _build()
    return _programs_cache["l"]


def _make_in_maps(C, A, D):
    in_maps = []
    for c in range(NCORES):
        s = 512 * c
        e = min(s + CR, NMAT)
        if e - s == CR:
            in_maps.append({"cb": C[s:e], "ab": A[s:e], "db": D[s:e]})
        else:
            m = {}
            for name, M in (("cb", C), ("ab", A), ("db", D)):
                t = np.zeros((CR, NMAT), dtype=np.float32)
                t[:e - s] = M[s:e]
                m[name] = t
            in_maps.append(m)
    return in_maps


def kernel(filtered_cell_kernel, filtered_drug_sim, original_cell_drug_adj,
           enable_homogeneous_graph):
    C = np.ascontiguousarray(np.asarray(filtered_cell_kernel, dtype=np.float32))
    D = np.ascontiguousarray(np.asarray(filtered_drug_sim, dtype=np.float32))
    A = np.ascontiguousarray(np.asarray(original_cell_drug_adj, dtype=np.float32))
    enable = int(np.asarray(enable_homogeneous_graph))
    if not enable:
        C = np.zeros_like(C)
        D = np.zeros_like(D)

    r = run_bass_kernel_spmd(_program(), _make_in_maps(C, A, D),
                             core_ids=list(range(NCORES))).results

    out = np.empty((N, N), dtype=np.float32)
    for c in range(NCORES):
        s = 512 * c
        e = min(s + CR, NMAT)
        n = e - s
        out[s:e, :] = r[c]["top"][:n]
        out[NMAT + s:NMAT + e, NMAT:] = r[c]["br"][:n]
        out[NMAT:, s:e] = r[c]["ats"][:, :n]
    idx = np.arange(N)
    out[idx, idx] += np.float32(1.0)
    return out
